# revision 1
# baseline (speedup 1.0000x reference)
"""Trainium2 Bass kernel for the GNN attention-head message-passing module.

Contract: kernel(**inputs) takes the FULL (unsharded) numpy inputs and
returns the FULL [N, C_OUT] float32 output, distributing work across 8
NeuronCores internally.

Math (reference):
    fts  = seq @ W_seq.T                      [N, CO]
    f1   = fts @ w_f1 + b_f1 ; f2 = fts @ w_f2 + b_f2
    e_e  = selu(f1[src_e] + f2[dst_e])        per edge
    coef = softmax(e) (global over edges)
    vals[n] = sum_{e:src=n} coef_e*fts[dst_e] + sum_{e:dst=n} coef_e*fts[n]
    out  = elu(vals + bias + seq @ W_res.T + b_res)

Device strategy (8 cores, SPMD single program):
  - softmax linearity: accumulate with raw exp weights, divide by the
    global Z = sum(exp) once at the end.  exp(selu(x)) is bounded for this
    distribution so no max-subtraction pass is needed.
  - nodes are assigned to cores round-robin over the degree-sorted order
    (separately for out-degree / src side and in-degree / dst side), which
    makes the ELL (chunk-of-128-nodes x pass) edge layout a shared
    compile-time constant across cores with ~2% slot padding.
  - src sweep (edges owned by their src's core): one indirect-DMA gather of
    the 516B table row [fts|f2] per edge, exp(selu(f1_seg + f2)) fused on
    ACT/DVE, per-edge scaling on DVE, and a scatter-free accumulation of
    U[n] = sum exp*fts[dst] via identity-weight matmuls into PSUM (edge
    slot p of a tile maps to node p of its chunk).
  - dst sweep: per-edge exp values are re-gathered (4B descriptors) from
    the AllGathered exp buffer in dst-ELL order; s_dst[n] = sum exp over
    in-edges reduces along the free axis with no scatter.
  - collectives: AllGather of the [NSH,129] table shard, of the exp buffer
    and of the s_dst partials.  Z is reduced locally from the gathered
    s partials (identical order on every core).
"""

import sys

if "/opt/trn_rl_repo" not in sys.path:
    sys.path.insert(0, "/opt/trn_rl_repo")

import numpy as np

P = 128
LAMBDA = 1.0507009873554805
ALPHA = 1.6732632423543772
MASK_NEG = -100.0


def _ceil_to(x, m):
    return ((x + m - 1) // m) * m


class Plan:
    """Host-side sharding plan + per-core input arrays (all numpy)."""

    def __init__(self, seq, edge_index, W_seq, w_f1, b_f1, w_f2, b_f2, bias,
                 W_res, b_res, R=8, B=40, Bd=64):
        N, C = seq.shape
        CO = W_seq.shape[0]
        E = edge_index.shape[1]
        assert C % P == 0 and CO <= P
        self.N, self.C, self.CO, self.E, self.R = N, C, CO, E, R
        self.B, self.Bd = B, Bd

        src = np.asarray(edge_index[0], dtype=np.int64)
        dst = np.asarray(edge_index[1], dtype=np.int64)

        npc = -(-N // R)                       # max nodes per core
        NCH = -(-npc // P)                     # chunks per core
        NSH = NCH * P
        self.NCH, self.NSH = NCH, NSH

        # ---------- src side (out-degree) ----------
        deg = np.bincount(src, minlength=N)
        order = np.argsort(-deg, kind="stable")
        rank_of = np.empty(N, np.int64)
        rank_of[order] = np.arange(N)
        core_of = (rank_of % R).astype(np.int64)
        lslot = (rank_of // R).astype(np.int64)
        degs_sorted = deg[order]
        p_src = [int(degs_sorted[min(k * P * R, N - 1)]) for k in range(NCH)]
        off_src = np.concatenate([[0], np.cumsum(p_src)]).astype(np.int64)
        T_src = int(off_src[-1])
        T_srcP = (T_src // B + 1) * B          # strictly > T_src
        self.p_src, self.off_src, self.T_src, self.T_srcP = p_src, off_src, T_src, T_srcP
        self.order, self.core_of, self.lslot = order, core_of, lslot

        es = np.argsort(src, kind="stable")
        starts = np.zeros(N + 1, np.int64)
        starts[1:] = np.cumsum(deg)
        epos = np.arange(E) - starts[src[es]]
        n_ = src[es]
        c_ = core_of[n_]
        pp = lslot[n_] % P
        kk = lslot[n_] // P
        t_ = off_src[kk] + epos
        row_ = core_of[dst[es]] * NSH + lslot[dst[es]]

        dst_idx = np.zeros((R, P, T_srcP), np.int32)
        maskm = np.full((R, P, T_srcP), MASK_NEG, np.float32)
        dst_idx[c_, pp, t_] = row_.astype(np.int32)
        maskm[c_, pp, t_] = 0.0
        self.dst_idx, self.maskm = dst_idx, maskm

        exp_pos = np.empty(E, np.int64)
        exp_pos[es] = (c_ * P + pp) * T_srcP + t_

        # ---------- dst side (in-degree) ----------
        degi = np.bincount(dst, minlength=N)
        orderi = np.argsort(-degi, kind="stable")
        ranki = np.empty(N, np.int64)
        ranki[orderi] = np.arange(N)
        corei = (ranki % R).astype(np.int64)
        lsloti = (ranki // R).astype(np.int64)
        degi_sorted = degi[orderi]
        p_dst = [int(degi_sorted[min(k * P * R, N - 1)]) for k in range(NCH)]
        off_dst = np.concatenate([[0], np.cumsum(p_dst)]).astype(np.int64)
        T_dst = int(off_dst[-1])
        T_dstP = max(_ceil_to(T_dst, Bd), Bd)
        self.p_dst, self.off_dst, self.T_dst, self.T_dstP = p_dst, off_dst, T_dst, T_dstP

        ed = np.argsort(dst, kind="stable")
        startsd = np.zeros(N + 1, np.int64)
        startsd[1:] = np.cumsum(degi)
        eposd = np.arange(E) - startsd[dst[ed]]
        n2 = dst[ed]
        c2 = corei[n2]
        p2 = lsloti[n2] % P
        k2 = lsloti[n2] // P
        t2 = off_dst[k2] + eposd

        pad_pos = T_srcP - 1                   # core 0, partition 0, tail pad tile
        pi_idx = np.full((R, P, T_dstP), pad_pos, np.int32)
        pi_idx[c2, p2, t2] = exp_pos[ed].astype(np.int32)
        self.pi_idx = pi_idx

        # ---------- s reorder indices (canonical src-order <- dst-order) ----------
        sidx = np.zeros((R, P, NCH), np.int32)
        for c in range(R):
            m = np.arange(NCH)[None, :]
            p = np.arange(P)[:, None]
            r = (m * P + p) * R + c            # canonical global rank
            valid = r < N
            x = order[np.minimum(r, N - 1)]
            flat = (corei[x] * P + (lsloti[x] % P)) * NCH + lsloti[x] // P
            sidx[c] = np.where(valid, flat, 0).astype(np.int32)
        self.sidx = sidx

        # ---------- per-core dense inputs ----------
        seq = np.asarray(seq, np.float32)
        self.seqT = np.zeros((R, C, NSH), np.float32)
        self.ncore_nodes = []
        for c in range(R):
            nodes_c = order[c::R]
            self.ncore_nodes.append(nodes_c)
            self.seqT[c, :, : len(nodes_c)] = seq[nodes_c].T

        W_seq = np.asarray(W_seq, np.float32)
        W_res = np.asarray(W_res, np.float32)
        w_f1 = np.asarray(w_f1, np.float32).reshape(CO)
        w_f2 = np.asarray(w_f2, np.float32).reshape(CO)
        u1 = W_seq.T @ w_f1
        u2 = W_seq.T @ w_f2
        self.wcat = np.concatenate(
            [W_seq.T, W_res.T, u1[:, None], u2[:, None]], axis=1
        ).astype(np.float32)                   # [C, 2*CO+2]
        self.brow = (np.asarray(bias, np.float32) + np.asarray(b_res, np.float32)
                     ).reshape(1, CO)
        self.b12 = float(np.asarray(b_f1, np.float32) + np.asarray(b_f2, np.float32))

    def in_maps(self):
        maps = []
        for c in range(self.R):
            maps.append({
                "seqT": np.ascontiguousarray(self.seqT[c]),
                "wcat": self.wcat,
                "dsti": np.ascontiguousarray(self.dst_idx[c]),
                "maskm": np.ascontiguousarray(self.maskm[c]),
                "pii": np.ascontiguousarray(self.pi_idx[c]),
                "sidx": np.ascontiguousarray(self.sidx[c]),
                "brow": self.brow,
            })
        return maps

    def unshard(self, results):
        out = np.empty((self.N, self.CO), np.float32)
        for c in range(self.R):
            nodes_c = self.ncore_nodes[c]
            out[nodes_c] = results[c]["out"][: len(nodes_c)]
        return out


def _segments(t0, t1, off, nch):
    """Chunk segments (k, lo, hi) covering tile range [t0, t1)."""
    segs = []
    for k in range(nch):
        lo = max(int(off[k]), t0)
        hi = min(int(off[k + 1]), t1)
        if lo < hi:
            segs.append((k, lo, hi))
    return segs


def build_program(plan, debug_outputs=False):
    import concourse.bacc as bacc
    import concourse.bass as bass
    import concourse.mybir as mybir
    import concourse.tile as tile
    from concourse.masks import make_identity

    f32 = mybir.dt.float32
    i32 = mybir.dt.int32
    Alu = mybir.AluOpType
    Act = mybir.ActivationFunctionType
    Ax = mybir.AxisListType
    IOA = bass.IndirectOffsetOnAxis

    R, C, CO = plan.R, plan.C, plan.CO
    NCH, NSH = plan.NCH, plan.NSH
    B, Bd = plan.B, plan.Bd
    T_srcP, T_dstP = plan.T_srcP, plan.T_dstP
    KC = C // P
    WN = 2 * CO + 2
    LA = LAMBDA * ALPHA

    nc = bacc.Bacc("TRN2", target_bir_lowering=False, debug=False, num_devices=R)

    seqT = nc.dram_tensor("seqT", [C, NSH], f32, kind="ExternalInput")
    wcat = nc.dram_tensor("wcat", [C, WN], f32, kind="ExternalInput")
    dsti = nc.dram_tensor("dsti", [P, T_srcP], i32, kind="ExternalInput")
    maskm = nc.dram_tensor("maskm", [P, T_srcP], f32, kind="ExternalInput")
    pii = nc.dram_tensor("pii", [P, T_dstP], i32, kind="ExternalInput")
    sidx = nc.dram_tensor("sidx", [P, NCH], i32, kind="ExternalInput")
    brow = nc.dram_tensor("brow", [1, CO], f32, kind="ExternalInput")
    out_sh = nc.dram_tensor("out", [NSH, CO], f32, kind="ExternalOutput")

    table_sh = nc.dram_tensor("table_sh", [NSH, CO + 1], f32)
    table = nc.dram_tensor("table", [R * NSH, CO + 1], f32)
    expb = nc.dram_tensor("expb", [P, T_srcP], f32)
    expf = nc.dram_tensor("expf", [R * P * T_srcP, 1], f32)
    s_sh = nc.dram_tensor("s_sh", [P, NCH], f32)
    s_f = nc.dram_tensor("s_f", [R * P * NCH, 1], f32)
    if debug_outputs:
        udbg = nc.dram_tensor("udbg", [NSH, CO], f32, kind="ExternalOutput")
        tdbg = nc.dram_tensor("tdbg", [NSH, CO + 1], f32, kind="ExternalOutput")
        edbg = nc.dram_tensor("edbg", [P, T_srcP], f32, kind="ExternalOutput")
        sdbg = nc.dram_tensor("sdbg", [P, NCH], f32, kind="ExternalOutput")
    else:
        udbg = tdbg = edbg = sdbg = None

    groups = [list(range(R))]
    n_sb = T_srcP // B
    n_db = T_dstP // Bd

    with tile.TileContext(nc) as tc:
        with tc.tile_pool(name="persist", bufs=1) as pp, \
             tc.tile_pool(name="work", bufs=4) as wp, \
             tc.tile_pool(name="psumU", bufs=4, space="PSUM") as pup:

            # persistent stashes
            tableS = pp.tile([P, NCH, CO + 1], f32)   # [slot, chunk, fts|f2]
            US = pp.tile([P, NCH, CO], f32)
            resS = pp.tile([P, NCH, CO], f32)
            f1S = pp.tile([P, NCH], f32)
            sS = pp.tile([P, NCH], f32)
            ident = pp.tile([P, P], f32)
            biasmat = pp.tile([P, CO], f32)
            onesRow = pp.tile([1, P], f32)
            onesCol = pp.tile([P, 1], f32)
            zinvC = pp.tile([P, 1], f32)
            sMineZ = pp.tile([P, NCH], f32)
            zeroB = pp.tile([P, 1], f32)
            negLA = pp.tile([P, 1], f32)
            nc.vector.memset(zeroB[:], 0.0)
            nc.vector.memset(negLA[:], -LA)

            make_identity(nc, ident[:])
            nc.vector.memset(onesRow[:], 1.0)
            nc.vector.memset(onesCol[:], 1.0)
            nc.vector.memset(sS[:], 0.0)

            # ---------------- phase 1: fts/f1/f2/res ----------------
            wcatS = pp.tile([P, KC, WN], f32)
            for k in range(KC):
                nc.sync.dma_start(out=wcatS[:, k, :], in_=wcat[k * P:(k + 1) * P, :])
            browS = pp.tile([1, CO], f32)
            nc.sync.dma_start(out=browS[:], in_=brow[:, :])

            with tc.tile_pool(name="ph1psum", bufs=2, space="PSUM") as p1p:
                for m in range(NCH):
                    lhs = wp.tile([P, KC, P], f32)
                    for k in range(KC):
                        nc.sync.dma_start(
                            out=lhs[:, k, :],
                            in_=seqT[k * P:(k + 1) * P, m * P:(m + 1) * P])
                    ps1 = p1p.tile([P, WN], f32)
                    for k in range(KC):
                        nc.tensor.matmul(out=ps1[:], lhsT=lhs[:, k, :],
                                         rhs=wcatS[:, k, :],
                                         start=(k == 0), stop=(k == KC - 1))
                    nc.vector.tensor_copy(out=tableS[:, m, 0:CO], in_=ps1[:, 0:CO])
                    nc.vector.tensor_copy(out=tableS[:, m, CO:CO + 1],
                                          in_=ps1[:, 2 * CO + 1:2 * CO + 2])
                    nc.vector.tensor_copy(out=resS[:, m, :], in_=ps1[:, CO:2 * CO])
                    nc.vector.tensor_copy(out=f1S[:, m:m + 1],
                                          in_=ps1[:, 2 * CO:2 * CO + 1])
                # bias broadcast matrix: ones^T @ brow
                psb = p1p.tile([P, CO], f32)
                nc.tensor.matmul(out=psb[:], lhsT=onesRow[:], rhs=browS[:],
                                 start=True, stop=True)
                nc.vector.tensor_copy(out=biasmat[:], in_=psb[:])

            nc.sync.dma_start(
                out=table_sh[:, :].rearrange("(m s) f -> s m f", s=P),
                in_=tableS[:])
            if tdbg is not None:
                nc.sync.dma_start(
                    out=tdbg[:, :].rearrange("(m s) f -> s m f", s=P),
                    in_=tableS[:])
            nc.gpsimd.collective_compute(
                "AllGather", Alu.bypass, replica_groups=groups,
                ins=[table_sh[:, :]], outs=[table[:, :]])

            # ---------------- phase 2: src sweep ----------------
            psum_tiles = {}
            for b in range(n_sb):
                t0, t1 = b * B, (b + 1) * B
                idxT = wp.tile([P, B], i32)
                nc.sync.dma_start(out=idxT[:], in_=dsti[:, t0:t1])
                G = wp.tile([P, B, CO + 1], f32)
                for j in range(B):
                    nc.gpsimd.indirect_dma_start(
                        out=G[:, j, :], out_offset=None, in_=table[:, :],
                        in_offset=IOA(ap=idxT[:, j:j + 1], axis=0))

                segs = _segments(t0, t1, plan.off_src, NCH)
                cov = sum(hi - lo for _, lo, hi in segs)
                if cov < t1 - t0:               # pad tiles -> pseudo segment
                    lo = t0 + cov
                    segs = segs + [(-1, lo, t1)]

                coefT = wp.tile([P, B], f32)
                for k, lo, hi in segs:
                    kk = max(k, 0)
                    nc.vector.tensor_scalar(
                        out=coefT[:, lo - t0:hi - t0],
                        in0=G[:, lo - t0:hi - t0, CO],
                        scalar1=f1S[:, kk:kk + 1], scalar2=plan.b12,
                        op0=Alu.add, op1=Alu.add)
                mmT = wp.tile([P, B], f32)
                nc.sync.dma_start(out=mmT[:], in_=maskm[:, t0:t1])
                rT = wp.tile([P, B], f32)
                mT = wp.tile([P, B], f32)
                uT = wp.tile([P, B], f32)
                nc.vector.tensor_scalar_max(out=rT[:], in0=coefT[:], scalar1=0.0)
                nc.vector.tensor_tensor(out=mT[:], in0=coefT[:], in1=rT[:],
                                        op=Alu.subtract)
                nc.vector.tensor_tensor(out=rT[:], in0=rT[:], in1=mmT[:],
                                        op=Alu.add)
                nc.scalar.activation(out=mT[:], in_=mT[:], func=Act.Exp,
                                     bias=zeroB[:])
                nc.scalar.activation(out=uT[:], in_=mT[:], func=Act.Exp,
                                     bias=negLA[:], scale=LA)
                nc.scalar.activation(out=rT[:], in_=rT[:], func=Act.Exp,
                                     bias=zeroB[:], scale=LAMBDA)
                nc.vector.tensor_tensor(out=coefT[:], in0=uT[:], in1=rT[:],
                                        op=Alu.mult)
                nc.sync.dma_start(out=expb[:, t0:t1], in_=coefT[:])
                if edbg is not None:
                    nc.sync.dma_start(out=edbg[:, t0:t1], in_=coefT[:])

                for k, lo, hi in segs:
                    if k < 0:
                        continue
                    for t in range(lo, hi):
                        j = t - t0
                        nc.vector.tensor_scalar_mul(
                            out=G[:, j, 0:CO], in0=G[:, j, 0:CO],
                            scalar1=coefT[:, j:j + 1])
                        if t == plan.off_src[k]:
                            psum_tiles[k] = pup.tile([P, CO], f32, name="psU", tag="psU")
                        last = (t == plan.off_src[k + 1] - 1)
                        nc.tensor.matmul(out=psum_tiles[k][:], lhsT=ident[:],
                                         rhs=G[:, j, 0:CO],
                                         start=(t == plan.off_src[k]), stop=last)
                        if last:
                            nc.vector.tensor_copy(out=US[:, k, :],
                                                  in_=psum_tiles[k][:])
                            del psum_tiles[k]
            for k in range(NCH):
                if plan.p_src[k] == 0:
                    nc.vector.memset(US[:, k, :], 0.0)
            if udbg is not None:
                nc.sync.dma_start(
                    out=udbg[:, :].rearrange("(m s) f -> s m f", s=P),
                    in_=US[:])

            nc.gpsimd.collective_compute(
                "AllGather", Alu.bypass, replica_groups=groups,
                ins=[expb[:, :]], outs=[expf[:, :]])

            # ---------------- phase 3: dst sweep (s_dst) ----------------
            for b in range(n_db):
                t0, t1 = b * Bd, (b + 1) * Bd
                piT = wp.tile([P, Bd], i32)
                nc.sync.dma_start(out=piT[:], in_=pii[:, t0:t1])
                exT = wp.tile([P, Bd], f32)
                for j in range(Bd):
                    nc.gpsimd.indirect_dma_start(
                        out=exT[:, j:j + 1], out_offset=None, in_=expf[:, :],
                        in_offset=IOA(ap=piT[:, j:j + 1], axis=0))
                for k, lo, hi in _segments(t0, t1, plan.off_dst, NCH):
                    if lo == plan.off_dst[k]:
                        nc.vector.tensor_reduce(
                            out=sS[:, k:k + 1], in_=exT[:, lo - t0:hi - t0],
                            axis=Ax.X, op=Alu.add)
                    else:
                        tmp = wp.tile([P, 1], f32)
                        nc.vector.tensor_reduce(
                            out=tmp[:], in_=exT[:, lo - t0:hi - t0],
                            axis=Ax.X, op=Alu.add)
                        nc.vector.tensor_tensor(out=sS[:, k:k + 1],
                                                in0=sS[:, k:k + 1],
                                                in1=tmp[:], op=Alu.add)
            nc.sync.dma_start(out=s_sh[:, :], in_=sS[:])
            if sdbg is not None:
                nc.sync.dma_start(out=sdbg[:, :], in_=sS[:])
            nc.gpsimd.collective_compute(
                "AllGather", Alu.bypass, replica_groups=groups,
                ins=[s_sh[:, :]], outs=[s_f[:, :]])

            # ---------------- phase 4: Z, s reorder, final ----------------
            with tc.tile_pool(name="zpsum", bufs=2, space="PSUM") as zp:
                zl = wp.tile([P, R, NCH], f32)
                nc.sync.dma_start(
                    out=zl[:],
                    in_=s_f[:, :].rearrange("(r p c) o -> p r (c o)", r=R, p=P))
                zpart = wp.tile([P, 1], f32)
                nc.vector.tensor_reduce(out=zpart[:], in_=zl[:], axis=Ax.XY,
                                        op=Alu.add)
                psz = zp.tile([1, 1], f32)
                nc.tensor.matmul(out=psz[:], lhsT=zpart[:], rhs=onesCol[:],
                                 start=True, stop=True)
                zsb = wp.tile([1, 1], f32)
                nc.vector.tensor_copy(out=zsb[:], in_=psz[:])
                zinv1 = wp.tile([1, 1], f32)
                nc.vector.reciprocal(out=zinv1[:], in_=zsb[:])
                psb2 = zp.tile([P, 1], f32)
                nc.tensor.matmul(out=psb2[:], lhsT=onesRow[:], rhs=zinv1[:],
                                 start=True, stop=True)
                nc.vector.tensor_copy(out=zinvC[:], in_=psb2[:])

            sxT = wp.tile([P, NCH], i32)
            nc.sync.dma_start(out=sxT[:], in_=sidx[:, :])
            sMine = wp.tile([P, NCH], f32)
            for j in range(NCH):
                nc.gpsimd.indirect_dma_start(
                    out=sMine[:, j:j + 1], out_offset=None, in_=s_f[:, :],
                    in_offset=IOA(ap=sxT[:, j:j + 1], axis=0))
            nc.vector.tensor_scalar(out=sMineZ[:], in0=sMine[:],
                                    scalar1=zinvC[:, 0:1], scalar2=None,
                                    op0=Alu.mult)

            for m in range(NCH):
                xT = wp.tile([P, CO], f32)
                uz = wp.tile([P, CO], f32)
                nc.vector.tensor_scalar(out=xT[:], in0=tableS[:, m, 0:CO],
                                        scalar1=sMineZ[:, m:m + 1], scalar2=None,
                                        op0=Alu.mult)
                nc.vector.tensor_scalar(out=uz[:], in0=US[:, m, :],
                                        scalar1=zinvC[:, 0:1], scalar2=None,
                                        op0=Alu.mult)
                nc.vector.tensor_tensor(out=xT[:], in0=xT[:], in1=uz[:], op=Alu.add)
                nc.vector.tensor_tensor(out=xT[:], in0=xT[:], in1=resS[:, m, :],
                                        op=Alu.add)
                nc.vector.tensor_tensor(out=xT[:], in0=xT[:], in1=biasmat[:],
                                        op=Alu.add)
                r2 = wp.tile([P, CO], f32)
                m2 = wp.tile([P, CO], f32)
                nc.vector.tensor_scalar_max(out=r2[:], in0=xT[:], scalar1=0.0)
                nc.vector.tensor_tensor(out=m2[:], in0=xT[:], in1=r2[:],
                                        op=Alu.subtract)
                nc.scalar.activation(out=m2[:], in_=m2[:], func=Act.Exp,
                                     bias=zeroB[:])
                nc.vector.tensor_tensor(out=m2[:], in0=m2[:], in1=r2[:], op=Alu.add)
                nc.vector.tensor_scalar(out=m2[:], in0=m2[:], scalar1=-1.0,
                                        scalar2=None, op0=Alu.add)
                nc.sync.dma_start(out=out_sh[m * P:(m + 1) * P, :], in_=m2[:])

    nc.compile()
    return nc


def prepare(**inputs):
    """Build plan + program. Returns (plan, nc, in_maps)."""
    plan = Plan(
        np.asarray(inputs["seq"]), np.asarray(inputs["edge_index"]),
        np.asarray(inputs["W_seq"]), np.asarray(inputs["w_f1"]),
        np.asarray(inputs["b_f1"]), np.asarray(inputs["w_f2"]),
        np.asarray(inputs["b_f2"]), np.asarray(inputs["bias"]),
        np.asarray(inputs["W_res"]), np.asarray(inputs["b_res"]))
    nc = build_program(plan)
    return plan, nc, plan.in_maps()


def kernel(**inputs):
    from concourse.bass_utils import run_bass_kernel_spmd
    plan, nc, in_maps = prepare(**inputs)
    res = run_bass_kernel_spmd(nc, in_maps, core_ids=list(range(plan.R)))
    return plan.unshard(res.results)

